# revision 12
# baseline (speedup 1.0000x reference)
"""GCN 3-layer block on 8 Trainium2 NeuronCores.

Strategy (data-parallel over the 32 graph replicas, 4 graphs/core):
  - The GCN aggregation  agg = A_hat @ h  (A_hat = D^-1/2 (Adj + 2I) D^-1/2,
    E=16K edges over L=2048 nodes) is computed as a DENSE bf16 matmul on the
    TensorEngine. A_hat^T is built once on the host (outside HW time),
    shipped replicated to every core, and reused by all 4 local graphs x 3
    layers. Self-loops are folded into A_hat's diagonal; conv biases cancel
    inside BatchNorm and are dropped.
  - Layer ordering minimizes aggregation width: L1 agg@64 -> W1; L2 agg@128
    -> W2; L3 W3 -> agg@64.
  - Two data layouts: LC = [node-tiles on partitions, channels free] feeds
    the aggregation matmuls (contraction over nodes); CL = [channels on
    partitions, nodes free] feeds the W matmuls and makes BatchNorm a
    per-partition affine (single ScalarE activation pass). PE transposes
    (via identity) convert between them where needed.
  - BatchNorm statistics: per-channel sum fused into the DVE PSUM drains
    (tensor_scalar accum_out), sumsq via tensor_tensor_reduce, AllReduce'd
    across the 8 cores (tiny [128,2] f32), then scale/shift applied fused
    with ReLU on ScalarE in [128, 2048] chunks.
  - A_hat^T is loaded as 4 separate SBUF tiles so the first aggregation
    tile only waits on the first 2MB DMA, not the whole 8MB.
"""

import numpy as np
import ml_dtypes

import concourse.bass as bass
import concourse.bacc as bacc
import concourse.mybir as mybir
import concourse.tile as tile
from concourse import masks
from concourse.bass_utils import run_bass_kernel_spmd

BF16 = ml_dtypes.bfloat16

# Problem constants (nn_GCN1dBlock: x [4,8,64,2048], E=16384)
B, NREP, C0, L = 4, 8, 64, 2048
G_TOTAL = B * NREP          # 32 graphs
N_CORES = 8
G = G_TOTAL // N_CORES      # 4 graphs per core
NT = L // 128               # 16 node tiles
N_ROWS = G_TOTAL * L        # BN reduction length (global)
EPS = 1e-5
FP32 = mybir.dt.float32
BF = mybir.dt.bfloat16
ADD = mybir.AluOpType.add
MUL = mybir.AluOpType.mult
SUB = mybir.AluOpType.subtract


def build_program():
    nc = bacc.Bacc(None, target_bir_lowering=False, num_devices=N_CORES)

    # I/O --------------------------------------------------------------
    # AT packed [j, 128p, mj, k, q]: 4 DRAM blocks of 4 dst-tiles each
    at_dram = nc.dram_tensor("at", [4, 128, 4, NT, 128], BF, kind="ExternalInput")
    h0_dram = nc.dram_tensor("h0", [128, NT, G, 64], BF, kind="ExternalInput")
    w1_dram = nc.dram_tensor("w1", [64, 128], BF, kind="ExternalInput")
    w2_dram = nc.dram_tensor("w2", [128, 128], BF, kind="ExternalInput")
    w3_dram = nc.dram_tensor("w3", [128, 64], BF, kind="ExternalInput")
    # bn params: columns = [g1, be1, g2, be2, g3, be3]
    bn_dram = nc.dram_tensor("bn", [128, 6], FP32, kind="ExternalInput")
    out_dram = nc.dram_tensor("out", [G, 64, L], FP32, kind="ExternalOutput")

    warm_in = nc.dram_tensor("warm_in", [128, 2], FP32)
    warm_out = nc.dram_tensor("warm_out", [128, 2], FP32, addr_space="Shared")
    stats_in = [nc.dram_tensor(f"stats_in{i}", [128, 2], FP32) for i in range(3)]
    stats_out = [
        nc.dram_tensor(f"stats_out{i}", [128, 2], FP32, addr_space="Shared")
        for i in range(3)
    ]

    with tile.TileContext(nc) as tc:
        with (
            tc.tile_pool(name="const", bufs=1) as constp,
            tc.tile_pool(name="work", bufs=1) as work,
            tc.tile_pool(name="outp", bufs=2) as outp,
            tc.tile_pool(name="stat", bufs=1) as statp,
            tc.tile_pool(name="junk", bufs=2) as junkp,
            tc.tile_pool(name="pa", bufs=2, space=bass.MemorySpace.PSUM) as pa,
            tc.tile_pool(name="pt", bufs=2, space=bass.MemorySpace.PSUM) as pt,
            tc.tile_pool(name="pw", bufs=4, space=bass.MemorySpace.PSUM) as pw,
        ):
            # ---- constants -------------------------------------------
            # ncfw collective warmup: first AllReduce pays ~25us of one-time
            # setup; burn it here, overlapped with the input DMAs / agg1.
            warm_sb = statp.tile([128, 2], FP32, tag="warm")
            nc.vector.memset(warm_sb[:], 0.0)
            nc.sync.dma_start(warm_in[:], warm_sb[:])
            for _ in range(2):
                nc.gpsimd.collective_compute(
                    "AllReduce", ADD,
                    replica_groups=[list(range(N_CORES))],
                    ins=[warm_in[:]],
                    outs=[warm_out[:]],
                )

            h0 = constp.tile([128, NT, G, 64], BF, tag="h0")
            nc.sync.dma_start(h0[:], h0_dram[:])

            # at4[j][p, mj, k, q] = AT chunk for dst tiles m = 4j+mj
            # (at0 on the ACT HWDGE queue so it runs parallel to h0 on SP)
            at4 = []
            for j in range(4):
                t = constp.tile([128, 4, NT, 128], BF, tag=f"at{j}")
                eng = nc.scalar if j == 0 else nc.sync
                eng.dma_start(t[:], at_dram[j])
                at4.append(t)

            def at_lhsT(m, k):
                return at4[m // 4][:, m % 4, k, :]

            ident = constp.tile([128, 128], BF, tag="ident")
            masks.make_identity(nc, ident[:])

            w1 = constp.tile([64, 128], BF, tag="w1")
            w2 = constp.tile([128, 128], BF, tag="w2")
            w3 = constp.tile([128, 64], BF, tag="w3")
            nc.scalar.dma_start(w1[:], w1_dram[:])
            nc.scalar.dma_start(w2[:], w2_dram[:])
            nc.scalar.dma_start(w3[:], w3_dram[:])
            bn = constp.tile([128, 6], FP32, tag="bn")
            nc.scalar.dma_start(bn[:], bn_dram[:])
            eps_t = constp.tile([128, 1], FP32, tag="eps")
            nc.gpsimd.memset(eps_t[:], EPS)

            def bn_finalize(layer, acc_sum, acc_sq, nacc, cpart, nacc_q=None):
                """Reduce stat accumulators, AllReduce, produce scale/shift."""
                pack = statp.tile([128, 2], FP32, tag=f"pack{layer}")
                if cpart < 128:
                    nc.vector.memset(pack[:], 0.0)
                nc.vector.tensor_reduce(
                    pack[:cpart, 0:1], acc_sum[:cpart, :nacc],
                    axis=mybir.AxisListType.X, op=ADD,
                )
                nc.vector.tensor_reduce(
                    pack[:cpart, 1:2], acc_sq[:cpart, : (nacc_q or nacc)],
                    axis=mybir.AxisListType.X, op=ADD,
                )
                nc.sync.dma_start(stats_in[layer][:], pack[:])
                nc.gpsimd.collective_compute(
                    "AllReduce", ADD,
                    replica_groups=[list(range(N_CORES))],
                    ins=[stats_in[layer][:]],
                    outs=[stats_out[layer][:]],
                )
                red = statp.tile([128, 2], FP32, tag=f"red{layer}")
                nc.sync.dma_start(red[:], stats_out[layer][:])

                mom = statp.tile([128, 4], FP32, tag=f"mom{layer}")
                # mom cols: 0=mean, 1=E[x^2], 2=var, 3=sqrt(var+eps)
                nc.vector.tensor_scalar(mom[:, 0:2], red[:, 0:2], 1.0 / N_ROWS,
                                        None, MUL)
                nc.vector.tensor_tensor(mom[:, 2:3], mom[:, 0:1], mom[:, 0:1], MUL)
                nc.vector.tensor_tensor(mom[:, 2:3], mom[:, 1:2], mom[:, 2:3], SUB)
                nc.scalar.activation(
                    mom[:, 3:4], mom[:, 2:3],
                    mybir.ActivationFunctionType.Sqrt, bias=eps_t[:],
                )
                ss = statp.tile([128, 3], FP32, tag=f"ss{layer}")
                # ss cols: 0=rsqrt, 1=scale, 2=shift
                nc.vector.reciprocal(ss[:, 0:1], mom[:, 3:4])
                nc.vector.tensor_tensor(
                    ss[:, 1:2], ss[:, 0:1], bn[:, 2 * layer : 2 * layer + 1], MUL
                )
                nc.vector.tensor_tensor(ss[:, 2:3], mom[:, 0:1], ss[:, 1:2], MUL)
                nc.vector.tensor_tensor(
                    ss[:, 2:3], bn[:, 2 * layer + 1 : 2 * layer + 2], ss[:, 2:3], SUB
                )
                return ss

            # ================= Layer 1 ================================
            # agg1 (var1): out_LC[dst, g*64] ; lhsT = AT chunk, rhs = h0
            agg1_lc = work.tile([128, NT, G * 64], BF, tag="agg_lc")
            for m in range(NT):
                ps = pa.tile([128, G * 64], FP32, tag="pa")
                for k in range(NT):
                    nc.tensor.matmul(
                        ps[:], at_lhsT(m, k), h0[:, k, :, :],
                        start=(k == 0), stop=(k == NT - 1),
                    )
                nc.vector.tensor_copy(agg1_lc[:, m, :], ps[:])

            # transpose agg1 -> CL [64, G, NT, 128]
            agg1_cl = work.tile([64, G, NT, 128], BF, tag="agg_cl")
            for g in range(G):
                for m0 in range(0, NT, 4):
                    pst = pt.tile([64, 4, 128], BF, tag="pt")
                    for j in range(4):
                        nc.tensor.transpose(
                            pst[:, j, :],
                            agg1_lc[:, m0 + j, g * 64 : (g + 1) * 64],
                            ident[:],
                        )
                    nc.vector.tensor_copy(agg1_cl[:, g, m0 : m0 + 4, :], pst[:])

            # W1: h1pre_CL [128, G, NT, 128]; DVE drain fuses channel sums,
            # tensor_tensor_reduce(psum*sbuf_bf16) accumulates sumsq
            h1pre = work.tile([128, G, NT, 128], BF, tag="hpre")
            acc1_s = statp.tile([128, 16], FP32, tag="acc1s")
            acc1_q = statp.tile([128, 16], FP32, tag="acc1q")
            col = 0
            for g in range(G):
                for m0 in range(0, NT, 4):
                    psw = pw.tile([128, 512], FP32, tag="pw")
                    nc.tensor.matmul(
                        psw[:], w1[:], agg1_cl[:, g, m0 : m0 + 4, :],
                        start=True, stop=True,
                    )
                    nc.vector.tensor_scalar(
                        h1pre[:, g, m0 : m0 + 4, :], psw[:], 0.0, None, ADD, ADD,
                        accum_out=acc1_s[:, col : col + 1],
                    )
                    sq_junk = junkp.tile([128, 512], BF, tag="junk")
                    nc.scalar.activation(
                        sq_junk[:], psw[:],
                        mybir.ActivationFunctionType.Square,
                        accum_out=acc1_q[:, col : col + 1],
                    )
                    col += 1

            ss1 = bn_finalize(0, acc1_s, acc1_q, 16, 128)

            # per graph: normalize+relu (one big ACT op) then transpose to LC
            h1_cl = work.tile([128, G, NT, 128], BF, tag="h_cl")
            h1_lc = work.tile([128, NT, G, 128], BF, tag="h_lc")
            for g in range(G):
                nc.scalar.activation(
                    h1_cl[:, g, :, :], h1pre[:, g, :, :],
                    mybir.ActivationFunctionType.Relu,
                    bias=ss1[:, 2:3], scale=ss1[:, 1:2],
                )
                for m0 in range(0, NT, 4):
                    pst = pt.tile([128, 4, 128], BF, tag="pt")
                    for j in range(4):
                        nc.tensor.transpose(
                            pst[:, j, :], h1_cl[:, g, m0 + j, :], ident[:]
                        )
                    for j in range(4):
                        nc.vector.tensor_copy(h1_lc[:, m0 + j, g, :], pst[:, j, :])

            # ================= Layer 2 ================================
            # agg2 (var2): out_CL [128, dst] ; lhsT = h1_lc chunk, rhs = AT
            agg2_cl = work.tile([128, G, NT, 128], BF, tag="agg_cl")
            for g in range(G):
                for n0 in range(0, NT, 4):
                    ps = pa.tile([128, 512], FP32, tag="pa")
                    for k in range(NT):
                        nc.tensor.matmul(
                            ps[:],
                            h1_lc[:, k, g, :],
                            at4[n0 // 4][:, :, k, :],
                            start=(k == 0), stop=(k == NT - 1),
                        )
                    nc.vector.tensor_copy(agg2_cl[:, g, n0 : n0 + 4, :], ps[:])

            # W2 + fused stats
            h2pre = work.tile([128, G, NT, 128], BF, tag="hpre")
            acc2_s = statp.tile([128, 16], FP32, tag="acc2s")
            acc2_q = statp.tile([128, 16], FP32, tag="acc2q")
            col = 0
            for g in range(G):
                for m0 in range(0, NT, 4):
                    psw = pw.tile([128, 512], FP32, tag="pw")
                    nc.tensor.matmul(
                        psw[:], w2[:], agg2_cl[:, g, m0 : m0 + 4, :],
                        start=True, stop=True,
                    )
                    nc.vector.tensor_scalar(
                        h2pre[:, g, m0 : m0 + 4, :], psw[:], 0.0, None, ADD, ADD,
                        accum_out=acc2_s[:, col : col + 1],
                    )
                    sq_junk = junkp.tile([128, 512], BF, tag="junk")
                    nc.scalar.activation(
                        sq_junk[:], psw[:],
                        mybir.ActivationFunctionType.Square,
                        accum_out=acc2_q[:, col : col + 1],
                    )
                    col += 1

            ss2 = bn_finalize(1, acc2_s, acc2_q, 16, 128)

            h2_cl = work.tile([128, G, NT, 128], BF, tag="h_cl")
            for g in range(G):
                nc.scalar.activation(
                    h2_cl[:, g, :, :], h2pre[:, g, :, :],
                    mybir.ActivationFunctionType.Relu,
                    bias=ss2[:, 2:3], scale=ss2[:, 1:2],
                )

            # ================= Layer 3 ================================
            # W3 first: h2w_lc [128, NT, G, 64] = h2 @ W3
            h2w_lc = work.tile([128, NT, G, 64], BF, tag="h_lc2")
            for g in range(G):
                for m0 in range(0, NT, 4):
                    psw = pw.tile([128, 4, 64], FP32, tag="pw")
                    for j in range(4):
                        nc.tensor.matmul(
                            psw[:, j, :], h2_cl[:, g, m0 + j, :], w3[:],
                            start=True, stop=True,
                        )
                    for j in range(4):
                        nc.vector.tensor_copy(h2w_lc[:, m0 + j, g, :], psw[:, j, :])

            # agg3 (var1): out_LC [dst, g*64]
            agg3_lc = work.tile([128, NT, G * 64], BF, tag="agg_lc")
            for m in range(NT):
                ps = pa.tile([128, G * 64], FP32, tag="pa")
                for k in range(NT):
                    nc.tensor.matmul(
                        ps[:], at_lhsT(m, k), h2w_lc[:, k, :, :],
                        start=(k == 0), stop=(k == NT - 1),
                    )
                nc.vector.tensor_copy(agg3_lc[:, m, :], ps[:])

            # transpose agg3 -> CL (= h3pre) with fused stats on DVE
            agg3_cl = work.tile([64, G, NT, 128], BF, tag="agg_cl2")
            acc3_s = statp.tile([64, 16], FP32, tag="acc3s")
            acc3_q = statp.tile([64, 16], FP32, tag="acc3q")
            col = 0
            for g in range(G):
                for m0 in range(0, NT, 4):
                    pst = pt.tile([64, 4, 128], BF, tag="pt")
                    for j in range(4):
                        nc.tensor.transpose(
                            pst[:, j, :],
                            agg3_lc[:, m0 + j, g * 64 : (g + 1) * 64],
                            ident[:],
                        )
                    nc.vector.tensor_scalar(
                        agg3_cl[:, g, m0 : m0 + 4, :], pst[:], 0.0, None, ADD, ADD,
                        accum_out=acc3_s[:, col : col + 1],
                    )
                    sq_junk = junkp.tile([128, 512], BF, tag="junk")
                    nc.scalar.activation(
                        sq_junk[:64, :], pst[:],
                        mybir.ActivationFunctionType.Square,
                        accum_out=acc3_q[:, col : col + 1],
                    )
                    col += 1

            ss3 = bn_finalize(2, acc3_s, acc3_q, 16, 64)

            # BN3 + relu -> fp32 output, DMA out per graph
            for g in range(G):
                h3 = outp.tile([64, NT, 128], FP32, tag="h3")
                nc.scalar.activation(
                    h3[:], agg3_cl[:, g, :, :],
                    mybir.ActivationFunctionType.Relu,
                    bias=ss3[:64, 2:3], scale=ss3[:64, 1:2],
                )
                nc.sync.dma_start(out_dram[g], h3[:])

    nc.compile()
    return nc


_NC_CACHE = {}


def get_program():
    if "nc" not in _NC_CACHE:
        _NC_CACHE["nc"] = build_program()
    return _NC_CACHE["nc"]


def host_prep(x, edge_index):
    """Build AT (dense normalized adjacency, transposed+tiled) and h0 shards."""
    src = np.asarray(edge_index[0], np.int64)
    dst = np.asarray(edge_index[1], np.int64)
    deg = np.zeros(L, np.float32)
    np.add.at(deg, dst, 1.0)
    deg += 2.0
    dis = deg ** -0.5
    A = np.zeros((L, L), np.float32)
    np.add.at(A, (dst, src), (dis[src] * dis[dst]).astype(np.float32))
    idx = np.arange(L)
    A[idx, idx] += 2.0 / deg
    AT = A.T  # [src, dst]
    # at_pack[j, p, mj, k, q] = AT[k*128+p, (4j+mj)*128+q]
    at_pack = np.ascontiguousarray(
        AT.reshape(NT, 128, 4, 4, 128).transpose(2, 1, 3, 0, 4)
    ).astype(BF16)

    # x: [B, NREP, C0, L] -> [G_TOTAL, C0, L]; h0 LC pack:
    # h0_all[p, k, g, c] = x[g, c, k*128+p]
    xg = np.asarray(x, np.float32).reshape(G_TOTAL, C0, L)
    h0_all = np.ascontiguousarray(
        xg.reshape(G_TOTAL, C0, NT, 128).transpose(3, 2, 0, 1)
    ).astype(BF16)  # [128, NT, G_TOTAL, C0]
    return at_pack, h0_all


def kernel(x, edge_index, W1, b1, g1, be1, W2, b2, g2, be2, W3, b3, g3, be3):
    at_pack, h0_all = host_prep(x, edge_index)

    w1 = np.asarray(W1, np.float32).astype(BF16)
    w2 = np.asarray(W2, np.float32).astype(BF16)
    w3 = np.asarray(W3, np.float32).astype(BF16)
    bn = np.zeros((128, 6), np.float32)
    bn[:128, 0] = np.asarray(g1, np.float32)
    bn[:128, 1] = np.asarray(be1, np.float32)
    bn[:128, 2] = np.asarray(g2, np.float32)
    bn[:128, 3] = np.asarray(be2, np.float32)
    bn[:64, 4] = np.asarray(g3, np.float32)
    bn[:64, 5] = np.asarray(be3, np.float32)

    nc = get_program()
    in_maps = []
    for c in range(N_CORES):
        in_maps.append(
            {
                "at": at_pack,
                "h0": np.ascontiguousarray(h0_all[:, :, c * G : (c + 1) * G, :]),
                "w1": w1,
                "w2": w2,
                "w3": w3,
                "bn": bn,
            }
        )
    res = run_bass_kernel_spmd(nc, in_maps, core_ids=list(range(N_CORES)))
    out = np.concatenate([res.results[c]["out"] for c in range(N_CORES)], axis=0)
    return out.astype(np.float32)


# revision 13
# speedup vs baseline: 1.0141x; 1.0141x over previous
"""GCN 3-layer block on 8 Trainium2 NeuronCores.

Strategy (data-parallel over the 32 graph replicas, 4 graphs/core):
  - The GCN aggregation  agg = A_hat @ h  (A_hat = D^-1/2 (Adj + 2I) D^-1/2,
    E=16K edges over L=2048 nodes) is computed as a DENSE bf16 matmul on the
    TensorEngine. A_hat^T is built once on the host (outside HW time),
    shipped replicated to every core, and reused by all 4 local graphs x 3
    layers. Self-loops are folded into A_hat's diagonal; conv biases cancel
    inside BatchNorm and are dropped.
  - Layer ordering minimizes aggregation width: L1 agg@64 -> W1; L2 agg@128
    -> W2; L3 W3 -> agg@64.
  - Two data layouts: LC = [node-tiles on partitions, channels free] feeds
    the aggregation matmuls (contraction over nodes); CL = [channels on
    partitions, nodes free] feeds the W matmuls and makes BatchNorm a
    per-partition affine (single ScalarE activation pass). PE transposes
    (via identity) convert between them where needed.
  - BatchNorm statistics: per-channel sum fused into the DVE PSUM drains
    (tensor_scalar accum_out), sumsq via tensor_tensor_reduce, AllReduce'd
    across the 8 cores (tiny [128,2] f32), then scale/shift applied fused
    with ReLU on ScalarE in [128, 2048] chunks.
  - A_hat^T is loaded as 4 separate SBUF tiles so the first aggregation
    tile only waits on the first 2MB DMA, not the whole 8MB.
"""

import numpy as np
import ml_dtypes

import concourse.bass as bass
import concourse.bacc as bacc
import concourse.mybir as mybir
import concourse.tile as tile
from concourse import masks
from concourse.bass_utils import run_bass_kernel_spmd

BF16 = ml_dtypes.bfloat16

# Problem constants (nn_GCN1dBlock: x [4,8,64,2048], E=16384)
B, NREP, C0, L = 4, 8, 64, 2048
G_TOTAL = B * NREP          # 32 graphs
N_CORES = 8
G = G_TOTAL // N_CORES      # 4 graphs per core
NT = L // 128               # 16 node tiles
N_ROWS = G_TOTAL * L        # BN reduction length (global)
EPS = 1e-5
FP32 = mybir.dt.float32
BF = mybir.dt.bfloat16
ADD = mybir.AluOpType.add
MUL = mybir.AluOpType.mult
SUB = mybir.AluOpType.subtract


def build_program():
    nc = bacc.Bacc(None, target_bir_lowering=False, num_devices=N_CORES)

    # I/O --------------------------------------------------------------
    # AT packed [j, 128p, mj, k, q]: 4 DRAM blocks of 4 dst-tiles each
    at_dram = nc.dram_tensor("at", [4, 128, 4, NT, 128], BF, kind="ExternalInput")
    h0_dram = nc.dram_tensor("h0", [128, NT, G, 64], BF, kind="ExternalInput")
    w1_dram = nc.dram_tensor("w1", [64, 128], BF, kind="ExternalInput")
    w2_dram = nc.dram_tensor("w2", [128, 128], BF, kind="ExternalInput")
    w3_dram = nc.dram_tensor("w3", [128, 64], BF, kind="ExternalInput")
    # bn params: columns = [g1, be1, g2, be2, g3, be3]
    bn_dram = nc.dram_tensor("bn", [128, 6], FP32, kind="ExternalInput")
    out_dram = nc.dram_tensor("out", [G, 64, L], FP32, kind="ExternalOutput")

    warm_in = nc.dram_tensor("warm_in", [128, 2], FP32)
    warm_out = nc.dram_tensor("warm_out", [128, 2], FP32, addr_space="Shared")
    stats_in = [nc.dram_tensor(f"stats_in{i}", [128, 2], FP32) for i in range(3)]
    stats_out = [
        nc.dram_tensor(f"stats_out{i}", [128, 2], FP32, addr_space="Shared")
        for i in range(3)
    ]

    with tile.TileContext(nc) as tc:
        with (
            tc.tile_pool(name="const", bufs=1) as constp,
            tc.tile_pool(name="work", bufs=1) as work,
            tc.tile_pool(name="outp", bufs=2) as outp,
            tc.tile_pool(name="stat", bufs=1) as statp,
            tc.tile_pool(name="junk", bufs=2) as junkp,
            tc.tile_pool(name="pa", bufs=2, space=bass.MemorySpace.PSUM) as pa,
            tc.tile_pool(name="pt", bufs=2, space=bass.MemorySpace.PSUM) as pt,
            tc.tile_pool(name="pw", bufs=4, space=bass.MemorySpace.PSUM) as pw,
        ):
            # ---- constants -------------------------------------------
            # ncfw collective warmup: first AllReduce pays ~25us of one-time
            # setup; burn it here, overlapped with the input DMAs / agg1.
            warm_sb = statp.tile([128, 2], FP32, tag="warm")
            nc.vector.memset(warm_sb[:], 0.0)
            nc.sync.dma_start(warm_in[:], warm_sb[:])
            for _ in range(2):
                nc.gpsimd.collective_compute(
                    "AllReduce", ADD,
                    replica_groups=[list(range(N_CORES))],
                    ins=[warm_in[:]],
                    outs=[warm_out[:]],
                )

            h0 = constp.tile([128, NT, G, 64], BF, tag="h0")
            nc.sync.dma_start(h0[:], h0_dram[:])

            # at4[j][p, mj, k, q] = AT chunk for dst tiles m = 4j+mj
            # (at0 on the ACT HWDGE queue so it runs parallel to h0 on SP)
            at4 = []
            for j in range(4):
                t = constp.tile([128, 4, NT, 128], BF, tag=f"at{j}")
                eng = nc.scalar if j == 0 else nc.sync
                eng.dma_start(t[:], at_dram[j])
                at4.append(t)

            def at_lhsT(m, k):
                return at4[m // 4][:, m % 4, k, :]

            ident = constp.tile([128, 128], BF, tag="ident")
            masks.make_identity(nc, ident[:])

            w1 = constp.tile([64, 128], BF, tag="w1")
            w2 = constp.tile([128, 128], BF, tag="w2")
            w3 = constp.tile([128, 64], BF, tag="w3")
            nc.scalar.dma_start(w1[:], w1_dram[:])
            nc.scalar.dma_start(w2[:], w2_dram[:])
            nc.scalar.dma_start(w3[:], w3_dram[:])
            bn = constp.tile([128, 6], FP32, tag="bn")
            nc.scalar.dma_start(bn[:], bn_dram[:])
            eps_t = constp.tile([128, 1], FP32, tag="eps")
            nc.gpsimd.memset(eps_t[:], EPS)

            def bn_finalize(layer, acc_sum, acc_sq, nacc, cpart, nacc_q=None):
                """Reduce stat accumulators, AllReduce, produce scale/shift."""
                pack = statp.tile([128, 2], FP32, tag=f"pack{layer}")
                if cpart < 128:
                    nc.vector.memset(pack[:], 0.0)
                nc.vector.tensor_reduce(
                    pack[:cpart, 0:1], acc_sum[:cpart, :nacc],
                    axis=mybir.AxisListType.X, op=ADD,
                )
                nc.vector.tensor_reduce(
                    pack[:cpart, 1:2], acc_sq[:cpart, : (nacc_q or nacc)],
                    axis=mybir.AxisListType.X, op=ADD,
                )
                nc.sync.dma_start(stats_in[layer][:], pack[:])
                nc.gpsimd.collective_compute(
                    "AllReduce", ADD,
                    replica_groups=[list(range(N_CORES))],
                    ins=[stats_in[layer][:]],
                    outs=[stats_out[layer][:]],
                )
                red = statp.tile([128, 2], FP32, tag=f"red{layer}")
                nc.sync.dma_start(red[:], stats_out[layer][:])

                mom = statp.tile([128, 4], FP32, tag=f"mom{layer}")
                # mom cols: 0=mean, 1=E[x^2], 2=var, 3=sqrt(var+eps)
                nc.vector.tensor_scalar(mom[:, 0:2], red[:, 0:2], 1.0 / N_ROWS,
                                        None, MUL)
                nc.vector.tensor_tensor(mom[:, 2:3], mom[:, 0:1], mom[:, 0:1], MUL)
                nc.vector.tensor_tensor(mom[:, 2:3], mom[:, 1:2], mom[:, 2:3], SUB)
                nc.scalar.activation(
                    mom[:, 3:4], mom[:, 2:3],
                    mybir.ActivationFunctionType.Sqrt, bias=eps_t[:],
                )
                ss = statp.tile([128, 3], FP32, tag=f"ss{layer}")
                # ss cols: 0=rsqrt, 1=scale, 2=shift
                nc.vector.reciprocal(ss[:, 0:1], mom[:, 3:4])
                nc.vector.tensor_tensor(
                    ss[:, 1:2], ss[:, 0:1], bn[:, 2 * layer : 2 * layer + 1], MUL
                )
                nc.vector.tensor_tensor(ss[:, 2:3], mom[:, 0:1], ss[:, 1:2], MUL)
                nc.vector.tensor_tensor(
                    ss[:, 2:3], bn[:, 2 * layer + 1 : 2 * layer + 2], ss[:, 2:3], SUB
                )
                return ss

            # ================= Layer 1 ================================
            # agg1 (var1): out_LC[dst, g*64] ; lhsT = AT chunk, rhs = h0
            agg1_lc = work.tile([128, NT, G * 64], BF, tag="agg_lc")
            for m in range(NT):
                ps = pa.tile([128, G * 64], FP32, tag="pa")
                for k in range(NT):
                    nc.tensor.matmul(
                        ps[:], at_lhsT(m, k), h0[:, k, :, :],
                        start=(k == 0), stop=(k == NT - 1),
                    )
                nc.vector.tensor_copy(agg1_lc[:, m, :], ps[:])

            # transpose agg1 -> CL [64, G, NT, 128]
            agg1_cl = work.tile([64, G, NT, 128], BF, tag="agg_cl")
            for g in range(G):
                for m0 in range(0, NT, 4):
                    pst = pt.tile([64, 4, 128], BF, tag="pt")
                    for j in range(4):
                        nc.tensor.transpose(
                            pst[:, j, :],
                            agg1_lc[:, m0 + j, g * 64 : (g + 1) * 64],
                            ident[:],
                        )
                    nc.vector.tensor_copy(agg1_cl[:, g, m0 : m0 + 4, :], pst[:])

            # W1: h1pre_CL [128, G, NT, 128]; DVE drain fuses channel sums,
            # tensor_tensor_reduce(psum*sbuf_bf16) accumulates sumsq
            h1pre = work.tile([128, G, NT, 128], BF, tag="hpre")
            acc1_s = statp.tile([128, 16], FP32, tag="acc1s")
            acc1_q = statp.tile([128, 16], FP32, tag="acc1q")
            col = 0
            for g in range(G):
                for m0 in range(0, NT, 4):
                    psw = pw.tile([128, 512], FP32, tag="pw")
                    nc.tensor.matmul(
                        psw[:], w1[:], agg1_cl[:, g, m0 : m0 + 4, :],
                        start=True, stop=True,
                    )
                    nc.vector.tensor_scalar(
                        h1pre[:, g, m0 : m0 + 4, :], psw[:], 0.0, None, ADD, ADD,
                        accum_out=acc1_s[:, col : col + 1],
                    )
                    col += 1
                sq_junk = junkp.tile([128, NT, 128], BF, tag="junk")
                nc.scalar.activation(
                    sq_junk[:], h1pre[:, g, :, :],
                    mybir.ActivationFunctionType.Square,
                    accum_out=acc1_q[:, g : g + 1],
                )

            ss1 = bn_finalize(0, acc1_s, acc1_q, 16, 128, nacc_q=G)

            # per graph: normalize+relu (one big ACT op) then transpose to LC
            h1_cl = work.tile([128, G, NT, 128], BF, tag="h_cl")
            h1_lc = work.tile([128, NT, G, 128], BF, tag="h_lc")
            for g in range(G):
                nc.scalar.activation(
                    h1_cl[:, g, :, :], h1pre[:, g, :, :],
                    mybir.ActivationFunctionType.Relu,
                    bias=ss1[:, 2:3], scale=ss1[:, 1:2],
                )
                for m0 in range(0, NT, 4):
                    pst = pt.tile([128, 4, 128], BF, tag="pt")
                    for j in range(4):
                        nc.tensor.transpose(
                            pst[:, j, :], h1_cl[:, g, m0 + j, :], ident[:]
                        )
                    for j in range(4):
                        nc.vector.tensor_copy(h1_lc[:, m0 + j, g, :], pst[:, j, :])

            # ================= Layer 2 ================================
            # agg2 (var2): out_CL [128, dst] ; lhsT = h1_lc chunk, rhs = AT
            agg2_cl = work.tile([128, G, NT, 128], BF, tag="agg_cl")
            for g in range(G):
                for n0 in range(0, NT, 4):
                    ps = pa.tile([128, 512], FP32, tag="pa")
                    for k in range(NT):
                        nc.tensor.matmul(
                            ps[:],
                            h1_lc[:, k, g, :],
                            at4[n0 // 4][:, :, k, :],
                            start=(k == 0), stop=(k == NT - 1),
                        )
                    nc.vector.tensor_copy(agg2_cl[:, g, n0 : n0 + 4, :], ps[:])

            # W2 + fused stats
            h2pre = work.tile([128, G, NT, 128], BF, tag="hpre")
            acc2_s = statp.tile([128, 16], FP32, tag="acc2s")
            acc2_q = statp.tile([128, 16], FP32, tag="acc2q")
            col = 0
            for g in range(G):
                for m0 in range(0, NT, 4):
                    psw = pw.tile([128, 512], FP32, tag="pw")
                    nc.tensor.matmul(
                        psw[:], w2[:], agg2_cl[:, g, m0 : m0 + 4, :],
                        start=True, stop=True,
                    )
                    nc.vector.tensor_scalar(
                        h2pre[:, g, m0 : m0 + 4, :], psw[:], 0.0, None, ADD, ADD,
                        accum_out=acc2_s[:, col : col + 1],
                    )
                    col += 1
                sq_junk = junkp.tile([128, NT, 128], BF, tag="junk")
                nc.scalar.activation(
                    sq_junk[:], h2pre[:, g, :, :],
                    mybir.ActivationFunctionType.Square,
                    accum_out=acc2_q[:, g : g + 1],
                )

            ss2 = bn_finalize(1, acc2_s, acc2_q, 16, 128, nacc_q=G)

            h2_cl = work.tile([128, G, NT, 128], BF, tag="h_cl")
            for g in range(G):
                nc.scalar.activation(
                    h2_cl[:, g, :, :], h2pre[:, g, :, :],
                    mybir.ActivationFunctionType.Relu,
                    bias=ss2[:, 2:3], scale=ss2[:, 1:2],
                )

            # ================= Layer 3 ================================
            # W3 first: h2w_lc [128, NT, G, 64] = h2 @ W3
            h2w_lc = work.tile([128, NT, G, 64], BF, tag="h_lc2")
            for g in range(G):
                for m0 in range(0, NT, 4):
                    psw = pw.tile([128, 4, 64], FP32, tag="pw")
                    for j in range(4):
                        nc.tensor.matmul(
                            psw[:, j, :], h2_cl[:, g, m0 + j, :], w3[:],
                            start=True, stop=True,
                        )
                    for j in range(4):
                        nc.vector.tensor_copy(h2w_lc[:, m0 + j, g, :], psw[:, j, :])

            # agg3 (var1): out_LC [dst, g*64]
            agg3_lc = work.tile([128, NT, G * 64], BF, tag="agg_lc")
            for m in range(NT):
                ps = pa.tile([128, G * 64], FP32, tag="pa")
                for k in range(NT):
                    nc.tensor.matmul(
                        ps[:], at_lhsT(m, k), h2w_lc[:, k, :, :],
                        start=(k == 0), stop=(k == NT - 1),
                    )
                nc.vector.tensor_copy(agg3_lc[:, m, :], ps[:])

            # transpose agg3 -> CL (= h3pre) with fused stats on DVE
            agg3_cl = work.tile([64, G, NT, 128], BF, tag="agg_cl2")
            acc3_s = statp.tile([64, 16], FP32, tag="acc3s")
            acc3_q = statp.tile([64, 16], FP32, tag="acc3q")
            col = 0
            for g in range(G):
                for m0 in range(0, NT, 4):
                    pst = pt.tile([64, 4, 128], BF, tag="pt")
                    for j in range(4):
                        nc.tensor.transpose(
                            pst[:, j, :],
                            agg3_lc[:, m0 + j, g * 64 : (g + 1) * 64],
                            ident[:],
                        )
                    nc.vector.tensor_scalar(
                        agg3_cl[:, g, m0 : m0 + 4, :], pst[:], 0.0, None, ADD, ADD,
                        accum_out=acc3_s[:, col : col + 1],
                    )
                    col += 1
                sq_junk = junkp.tile([128, NT, 128], BF, tag="junk")
                nc.scalar.activation(
                    sq_junk[:64, :, :], agg3_cl[:, g, :, :],
                    mybir.ActivationFunctionType.Square,
                    accum_out=acc3_q[:, g : g + 1],
                )

            ss3 = bn_finalize(2, acc3_s, acc3_q, 16, 64, nacc_q=G)

            # BN3 + relu -> fp32 output, DMA out per graph
            for g in range(G):
                h3 = outp.tile([64, NT, 128], FP32, tag="h3")
                nc.scalar.activation(
                    h3[:], agg3_cl[:, g, :, :],
                    mybir.ActivationFunctionType.Relu,
                    bias=ss3[:64, 2:3], scale=ss3[:64, 1:2],
                )
                nc.sync.dma_start(out_dram[g], h3[:])

    nc.compile()
    return nc


_NC_CACHE = {}


def get_program():
    if "nc" not in _NC_CACHE:
        _NC_CACHE["nc"] = build_program()
    return _NC_CACHE["nc"]


def host_prep(x, edge_index):
    """Build AT (dense normalized adjacency, transposed+tiled) and h0 shards."""
    src = np.asarray(edge_index[0], np.int64)
    dst = np.asarray(edge_index[1], np.int64)
    deg = np.zeros(L, np.float32)
    np.add.at(deg, dst, 1.0)
    deg += 2.0
    dis = deg ** -0.5
    A = np.zeros((L, L), np.float32)
    np.add.at(A, (dst, src), (dis[src] * dis[dst]).astype(np.float32))
    idx = np.arange(L)
    A[idx, idx] += 2.0 / deg
    AT = A.T  # [src, dst]
    # at_pack[j, p, mj, k, q] = AT[k*128+p, (4j+mj)*128+q]
    at_pack = np.ascontiguousarray(
        AT.reshape(NT, 128, 4, 4, 128).transpose(2, 1, 3, 0, 4)
    ).astype(BF16)

    # x: [B, NREP, C0, L] -> [G_TOTAL, C0, L]; h0 LC pack:
    # h0_all[p, k, g, c] = x[g, c, k*128+p]
    xg = np.asarray(x, np.float32).reshape(G_TOTAL, C0, L)
    h0_all = np.ascontiguousarray(
        xg.reshape(G_TOTAL, C0, NT, 128).transpose(3, 2, 0, 1)
    ).astype(BF16)  # [128, NT, G_TOTAL, C0]
    return at_pack, h0_all


def kernel(x, edge_index, W1, b1, g1, be1, W2, b2, g2, be2, W3, b3, g3, be3):
    at_pack, h0_all = host_prep(x, edge_index)

    w1 = np.asarray(W1, np.float32).astype(BF16)
    w2 = np.asarray(W2, np.float32).astype(BF16)
    w3 = np.asarray(W3, np.float32).astype(BF16)
    bn = np.zeros((128, 6), np.float32)
    bn[:128, 0] = np.asarray(g1, np.float32)
    bn[:128, 1] = np.asarray(be1, np.float32)
    bn[:128, 2] = np.asarray(g2, np.float32)
    bn[:128, 3] = np.asarray(be2, np.float32)
    bn[:64, 4] = np.asarray(g3, np.float32)
    bn[:64, 5] = np.asarray(be3, np.float32)

    nc = get_program()
    in_maps = []
    for c in range(N_CORES):
        in_maps.append(
            {
                "at": at_pack,
                "h0": np.ascontiguousarray(h0_all[:, :, c * G : (c + 1) * G, :]),
                "w1": w1,
                "w2": w2,
                "w3": w3,
                "bn": bn,
            }
        )
    res = run_bass_kernel_spmd(nc, in_maps, core_ids=list(range(N_CORES)))
    out = np.concatenate([res.results[c]["out"] for c in range(N_CORES)], axis=0)
    return out.astype(np.float32)
